# revision 16
# baseline (speedup 1.0000x reference)
"""AnchorGenerator kernel for 8 TRN2 NeuronCores.

Output anchors[(k, fy, fx), 4] with x1,y1,x2,y2 = cx[fx]-w2[k], cy[fy]-h2[k],
cx[fx]+w2[k], cy[fy]+h2[k].  The feature_map VALUES are unused (only its
static shape matters), so only a 9 KB per-core column table is shipped.

Per core (fh sharded 8-ways, 128 rows each):
  - GpSimd iota generates cx = 8*fx+4 as exact f32 into a [128,1024] tile.
  - VectorE writes c=0,2 (cx +- w2[k], compile-time immediates) and c=3
    (0*cx + ycol) of each [128, 4096] slab; ScalarE writes c=1 via
    activation(Identity, scale=0, bias=ycol).  ycols holds cy[fy]-+h2[k]
    precomputed bit-exactly on the host.
  - HWDGE DMAs stream slabs to DRAM.  Slab 0 goes out as four 512 KB
    quarters so the stream starts ~3us earlier; the stream then runs at
    the 8-core HBM fair-share rate (~53us for 18.9 MB).
Raw Bass with explicit semaphores: this walrus build allows only ONE
sync-wait per instruction, so every wait is a standalone wait_ge.
"""

import sys

if "/opt/trn_rl_repo" not in sys.path:
    sys.path.insert(0, "/opt/trn_rl_repo")

import numpy as np

SCALES = (8.0, 16.0, 32.0)
RATIOS = (0.5, 1.0, 2.0)
STRIDE = 8.0
FH = 1024
FW = 1024
K = 9
N_CORES = 8
FH_LOC = FH // N_CORES  # 128 rows per core
ROW = FW * 4  # 4096 floats per (k, fy) row
NQ = 4  # slab 0 split into NQ quarter-DMAs
QW = FW // NQ  # x-range per quarter


def _anchor_consts():
    scales = np.asarray(SCALES, np.float32)
    sqrt_r = np.sqrt(np.asarray(RATIOS, np.float32)).astype(np.float32)
    ws = (scales[:, None] * sqrt_r[None, :]).reshape(-1).astype(np.float32)
    hs = (scales[:, None] / sqrt_r[None, :]).reshape(-1).astype(np.float32)
    w2 = (ws / np.float32(2.0)).astype(np.float32)
    h2 = (hs / np.float32(2.0)).astype(np.float32)
    return w2, h2


def _build_bass(final_wait=True, split_iota=None):
    import os

    import concourse.bass as bass
    import concourse.mybir as mybir

    if split_iota is None:
        split_iota = os.environ.get("ANCHOR_SPLIT_IOTA", "1") == "1"

    f32 = mybir.dt.float32
    w2, h2 = _anchor_consts()

    nc = bass.Bass()
    ycols = nc.dram_tensor("ycols", [FH_LOC, 2 * K], f32, kind="ExternalInput")
    out = nc.dram_tensor("out", [K * FH_LOC, ROW], f32, kind="ExternalOutput")

    with (
        nc.sbuf_tensor([FH_LOC, FW], f32) as B2,
        nc.sbuf_tensor([FH_LOC, 2 * K], f32) as ysb,
        nc.sbuf_tensor([FH_LOC, 1], f32) as scratch,
        nc.sbuf_tensor([FH_LOC, K * ROW], f32) as big,
        nc.semaphore() as in_sem,
        nc.semaphore() as g_sem,
        nc.semaphore() as v_sem,
        nc.semaphore() as a_sem,
        nc.semaphore() as o_sem,
        nc.Block() as block,
    ):
        big3 = big[:, :].rearrange("p (k x c) -> p k x c", k=K, c=4)
        mult = mybir.AluOpType.mult
        add = mybir.AluOpType.add
        ident = mybir.ActivationFunctionType.Identity

        def ycol(j):
            return ysb[:, j : j + 1]

        # Producer progress units: slab 0 counts NQ units, slabs 1.. one.
        @block.sync
        def _(sync):
            sync.dma_start(out=ysb[:, :], in_=ycols[:, :]).then_inc(in_sem, 16)
            n_dma = 0
            for q in range(NQ):
                sync.wait_ge(v_sem, q + 1)
                sync.wait_ge(a_sem, q + 1)
                sync.dma_start(
                    out=out[0:FH_LOC, q * QW * 4 : (q + 1) * QW * 4],
                    in_=big[:, q * QW * 4 : (q + 1) * QW * 4],
                ).then_inc(o_sem, 16)
                n_dma += 1
            for k in range(1, K):
                sync.wait_ge(v_sem, NQ + k)
                sync.wait_ge(a_sem, NQ + k)
                sync.dma_start(
                    out=out[k * FH_LOC : (k + 1) * FH_LOC, :],
                    in_=big[:, k * ROW : (k + 1) * ROW],
                ).then_inc(o_sem, 16)
                n_dma += 1
            if final_wait:
                sync.wait_ge(o_sem, 16 * n_dma)

        @block.gpsimd
        def _(g):
            if split_iota:
                # Split iota: first quarter-width chunk lands earlier so
                # slab 0's first quarter (and the DMA stream) starts sooner.
                nc.gpsimd.iota(
                    B2[:, 0:QW],
                    pattern=[[8, QW]],
                    base=4,
                    channel_multiplier=0,
                    allow_small_or_imprecise_dtypes=True,
                ).then_inc(g_sem, 1)
                nc.gpsimd.iota(
                    B2[:, QW:FW],
                    pattern=[[8, FW - QW]],
                    base=4 + 8 * QW,
                    channel_multiplier=0,
                    allow_small_or_imprecise_dtypes=True,
                ).then_inc(g_sem, 1)
            else:
                nc.gpsimd.iota(
                    B2[:, :],
                    pattern=[[8, FW]],
                    base=4,
                    channel_multiplier=0,
                    allow_small_or_imprecise_dtypes=True,
                ).then_inc(g_sem, 2)

        @block.vector
        def _(vector):
            vector.wait_ge(g_sem, 1)
            xs0 = slice(0, QW)
            nc.vector.tensor_scalar_add(
                big3[:, 0, xs0, 0], B2[:, xs0], float(-w2[0])
            )
            nc.vector.tensor_scalar_add(
                big3[:, 0, xs0, 2], B2[:, xs0], float(w2[0])
            )
            vector.wait_ge(in_sem, 16)
            nc.vector.tensor_scalar(
                big3[:, 0, xs0, 3], B2[:, xs0], 0.0, ycol(1), mult, add
            ).then_inc(v_sem, 1)
            vector.wait_ge(g_sem, 2)
            for q in range(1, NQ):
                xs = slice(q * QW, (q + 1) * QW)
                nc.vector.tensor_scalar_add(
                    big3[:, 0, xs, 0], B2[:, xs], float(-w2[0])
                )
                nc.vector.tensor_scalar_add(
                    big3[:, 0, xs, 2], B2[:, xs], float(w2[0])
                )
                nc.vector.tensor_scalar(
                    big3[:, 0, xs, 3], B2[:, xs], 0.0, ycol(1), mult, add
                ).then_inc(v_sem, 1)
            for k in range(1, K):
                nc.vector.tensor_scalar_add(
                    big3[:, k, :, 0], B2[:, :], float(-w2[k])
                )
                nc.vector.tensor_scalar_add(
                    big3[:, k, :, 2], B2[:, :], float(w2[k])
                )
                nc.vector.tensor_scalar(
                    big3[:, k, :, 3], B2[:, :], 0.0, ycol(2 * k + 1), mult, add
                ).then_inc(v_sem, 1)

        @block.scalar
        def _(s):
            # Dummy op preloads the Identity ACT table before deps resolve.
            nc.scalar.activation(
                scratch[:, 0:1], scratch[:, 0:1], ident, bias=0.0, scale=0.0
            )
            s.wait_ge(in_sem, 16)
            s.wait_ge(g_sem, 1)
            xs0 = slice(0, QW)
            nc.scalar.activation(
                big3[:, 0, xs0, 1], B2[:, xs0], ident, bias=ycol(0), scale=0.0
            ).then_inc(a_sem, 1)
            s.wait_ge(g_sem, 2)
            for q in range(1, NQ):
                xs = slice(q * QW, (q + 1) * QW)
                nc.scalar.activation(
                    big3[:, 0, xs, 1], B2[:, xs], ident, bias=ycol(0), scale=0.0
                ).then_inc(a_sem, 1)
            for k in range(1, K):
                nc.scalar.activation(
                    big3[:, k, :, 1], B2[:, :], ident, bias=ycol(2 * k), scale=0.0
                ).then_inc(a_sem, 1)

    return nc


def _host_inputs():
    """Per-core input: ycols[p, 2k+j] = cy[m*128+p] -+ h2[k]  (9 KB)."""
    _, h2 = _anchor_consts()
    cy = (np.arange(FH, dtype=np.float32) + np.float32(0.5)) * np.float32(STRIDE)
    in_maps = []
    for m in range(N_CORES):
        cym = cy[m * FH_LOC : (m + 1) * FH_LOC]
        yc = np.empty((FH_LOC, 2 * K), np.float32)
        for k in range(K):
            yc[:, 2 * k] = cym - h2[k]
            yc[:, 2 * k + 1] = cym + h2[k]
        in_maps.append({"ycols": yc})
    return in_maps


def run_spmd(trace=False, final_wait=True):
    """Build, compile and run the SPMD kernel on cores 0-7."""
    from concourse.bass_utils import run_bass_kernel_spmd

    nc = _build_bass(final_wait=final_wait)
    in_maps = _host_inputs()
    return run_bass_kernel_spmd(
        nc, in_maps, core_ids=list(range(N_CORES)), trace=trace
    )


def _assemble(results):
    full = np.empty((K, FH, ROW), np.float32)
    for m in range(N_CORES):
        full[:, m * FH_LOC : (m + 1) * FH_LOC, :] = np.asarray(
            results[m]["out"], dtype=np.float32
        ).reshape(K, FH_LOC, ROW)
    return full.reshape(-1, 4)


def kernel(feature_map=None, image_h=None, image_w=None, **_unused):
    res = run_spmd(trace=False)
    return _assemble(res.results)


if __name__ == "__main__":
    out = kernel()
    print(out.shape, out.dtype)
    print(out[:3])
